# revision 4
# baseline (speedup 1.0000x reference)
"""Trainium2 Bass kernel for segmented-LoRA linear (nn_Linear_73959336837249).

Math: out = x @ W.T + scale_g * ((x_g @ A_g.T) @ B_g.T), where the 16384
tokens form 4 contiguous segments of 4096, one adapter per segment.

Strategy:
  * Fold the LoRA update into the base weight per adapter on the host:
        Weff_g = W + s_g * B_g @ A_g        (exact algebraic identity)
    so each token segment needs a single dense matmul x_g @ Weff_g.T.
  * Shard tokens across the 8 NeuronCores (2048 tokens/core); each core's
    token range lives entirely inside one adapter segment, so each core
    gets exactly one [2048, 2048] effective weight (replicated per pair).
  * On device: one big [2048 x 2048] @ [2048 x 2048] fp32r matmul per core,
    K-tiled over PSUM with all 8 banks in flight.

Self-contained: hardcodes all shapes; no file I/O.
"""

import numpy as np

# Problem shapes (hardcoded per contest contract)
N_ADAPTERS = 4
RANK = 16
D_IN = 2048
D_OUT = 2048
TOKENS = 16384
N_CORES = 8

T_LOC = TOKENS // N_CORES  # 2048 tokens per core
P = 128                    # partitions
KT = D_IN // P             # 16 contraction tiles
TT = T_LOC // P            # 16 token tiles per core
ON = 512                   # output-column tile (one PSUM bank of fp32)
NO = D_OUT // ON           # 4 o-tiles

_NC = None


def _build_nc():
    import concourse.mybir as mybir
    import concourse.tile as tile
    from concourse import bacc

    fp32 = mybir.dt.float32
    fp32r = mybir.dt.float32r

    nc = bacc.Bacc(None, target_bir_lowering=False)

    # xt[t, p, k*128+j] = x_tok[t*128+j, k*128+p]  (token-tile-major, d on partitions)
    xt = nc.dram_tensor("xt", [TT, P, KT * P], fp32r, kind="ExternalInput")
    # wt[k, p, o] = Weff.T[k*128+p, o]
    wt = nc.dram_tensor("wt", [KT, P, D_OUT], fp32r, kind="ExternalInput")
    out = nc.dram_tensor("out", [T_LOC, D_OUT], fp32, kind="ExternalOutput")

    with tile.TileContext(nc) as tc:
        with (
            tc.tile_pool(name="wpool", bufs=1) as wpool,
            tc.tile_pool(name="xpool", bufs=3) as xpool,
            tc.tile_pool(name="opool", bufs=2) as opool,
            tc.tile_pool(name="pspool", bufs=2, space="PSUM") as pspool,
        ):
            # Resident effective weight: 16 tiles x [128, 2048] fp32r = 128 KB/partition
            w_tiles = []
            for k in range(KT):
                w_k = wpool.tile([P, D_OUT], fp32r, tag=f"w{k}", name=f"w_{k}")
                nc.sync.dma_start(w_k[:], wt[k])
                w_tiles.append(w_k)

            for t in range(TT):
                x_t = xpool.tile([P, KT * P], fp32r, tag="x", name=f"x_{t}")
                # gpsimd (SWDGE): engine-side sem waits, avoids the 2-wait
                # cap of HWDGE descriptor DMAs (slot-recycled loads carry 3)
                nc.gpsimd.dma_start(x_t[:], xt[t])
                o_t = opool.tile([P, D_OUT], fp32, tag="o", name=f"o_{t}")
                ps = [
                    pspool.tile([P, ON], fp32, tag=f"ps{o}", name=f"ps_{t}_{o}")
                    for o in range(NO)
                ]
                for k in range(KT):
                    lhsT = x_t[:, k * P:(k + 1) * P]  # [d=128, tok=128] stationary
                    for o in range(NO):
                        nc.tensor.matmul(
                            ps[o][:],
                            lhsT,
                            w_tiles[k][:, o * ON:(o + 1) * ON],
                            start=(k == 0),
                            stop=(k == KT - 1),
                        )
                for o in range(NO):
                    nc.vector.tensor_copy(o_t[:, o * ON:(o + 1) * ON], ps[o][:])
                nc.sync.dma_start(out[t * P:(t + 1) * P, :], o_t[:])

    nc.compile()
    return nc


def _get_nc():
    global _NC
    if _NC is None:
        _NC = _build_nc()
    return _NC


def _prep_inputs(inputs):
    x = np.ascontiguousarray(np.asarray(inputs["x"], dtype=np.float32))
    W = np.asarray(inputs["W"], dtype=np.float32)
    lora_a = np.asarray(inputs["lora_a"], dtype=np.float32)
    lora_b = np.asarray(inputs["lora_b"], dtype=np.float32)
    scalings = np.asarray(inputs["scalings"], dtype=np.float32)

    # Fold LoRA into the transposed effective weight per adapter:
    # Weff.T = W.T + s * A.T @ B.T  -> [d_in, d_out], tiled [KT, 128, D_OUT]
    wts = []
    for g in range(N_ADAPTERS):
        weff_t = W.T + scalings[g] * (lora_a[g].T @ lora_b[g].T)
        wts.append(
            np.ascontiguousarray(weff_t, dtype=np.float32).reshape(KT, P, D_OUT)
        )

    in_maps = []
    for c in range(N_CORES):
        xs = x[c * T_LOC:(c + 1) * T_LOC]  # [2048 tok, 2048 d]
        # [t, j, k, p] -> [t, p, k, j] -> [TT, 128, KT*128]
        xtl = np.ascontiguousarray(
            xs.reshape(TT, P, KT, P).transpose(0, 3, 2, 1)
        ).reshape(TT, P, KT * P)
        in_maps.append({"xt": xtl, "wt": wts[c * T_LOC // (TOKENS // N_ADAPTERS)]})
    return in_maps


def _run(inputs, trace=False, **kwargs):
    from concourse.bass_utils import run_bass_kernel_spmd

    nc = _get_nc()
    in_maps = _prep_inputs(inputs)
    res = run_bass_kernel_spmd(
        nc, in_maps, core_ids=list(range(N_CORES)), trace=trace, **kwargs
    )
    out = np.concatenate([r["out"] for r in res.results], axis=0)
    return out, res


def kernel(**inputs):
    out, _ = _run(inputs, trace=False)
    return out
